# revision 5
# baseline (speedup 1.0000x reference)
"""Trainium2 Bass kernel for 2-layer hetero GNN + MLP decoder — device-gather design.

Single NEFF, single launch per call:
  - nodes dealt to 8 cores by degree-sorted global tiles (tile g -> core g%8),
    shared padded-K slot schedule per 14-tile group;
  - device AllGather replicates x (and later h1) into per-core DRAM tables;
  - per-edge source rows fetched with indirect (dynamic-descriptor) DMA
    gathers into a GAPPED SBUF layout (one descriptor per slot);
  - DVE grouped segmented reduce -> per-tile PE transpose + matmul
    (weights stacked [tp | int | self+res | bias]);
  - layer2 + decoder fused via reversed matmuls (h2T = W^T @ UT2).
Host per call: bf16-cast x, content-hash inputs to skip re-uploads,
single-sync async pipeline (put -> exec -> fetch), unpermute [N,1] output.
"""
import sys
import hashlib

sys.path.insert(0, '/opt/trn_rl_repo')

import numpy as np
import ml_dtypes

import jax
from jax.sharding import Mesh, PartitionSpec, NamedSharding
from jax.experimental.shard_map import shard_map

import concourse.bass as bass
import concourse.bacc as bacc
import concourse.mybir as mybir
from concourse.tile import TileContext
from concourse.masks import make_identity
from concourse.bass2jax import _bass_exec_p, partition_id_tensor, install_neuronx_cc_hook

N_NODES = 100000
N_EDGES = 1600000
CIN, COUT = 6, 32
NCORES = 8
NT = 98                     # slot tiles per core
NPC = NT * 128              # padded nodes per core (12544)
NSH = N_NODES // NCORES     # x shard rows (12500)
GT = 7                      # tiles per group
NG = NT // GT               # 14 groups
WA = CIN                    # slot width, layer1 (bf16; no gap in column mode)
WB = COUT                   # slot width, layer2 (bf16)
XPAD = N_NODES              # zero row index in x table
HPAD = NCORES * NPC         # zero row index in h table (100352)
BF16 = ml_dtypes.bfloat16

_CACHE = {}
GATHER_MODE = "column"   # "group" (fast, needs multi-run pairing) | "column" (verified)


def _emit_gather(nc, pool, out_tile, w, c_use, idx_sb, ncols, table):
    """Gather rows of `table` into gapped out_tile [128, ncols*w] using
    idx_sb [128, ncols]; write [:, :, 0:c_use] of each slot."""
    if GATHER_MODE == "group":
        nc.gpsimd.indirect_dma_start(
            out=out_tile[:].rearrange("p (s w) -> p s w", w=w)[:, :, 0:c_use],
            out_offset=None, in_=table[:, :],
            in_offset=bass.IndirectOffsetOnAxis(ap=idx_sb[:], axis=0))
    else:
        v = out_tile[:].rearrange("p (s w) -> p s w", w=w)
        for k in range(ncols):
            nc.gpsimd.indirect_dma_start(
                out=v[:, k, 0:c_use],
                out_offset=None, in_=table[:, :],
                in_offset=bass.IndirectOffsetOnAxis(ap=idx_sb[:, k:k + 1], axis=0))


class _Compiled:
    """Compile-once PJRT executor for one Bass module on 8 cores."""

    def __init__(self, nc, n_cores):
        install_neuronx_cc_hook()
        self.nc = nc
        self.n_cores = n_cores
        pname = nc.partition_id_tensor.name if nc.partition_id_tensor else None
        in_names, out_names, out_avals = [], [], []
        for alloc in nc.m.functions[0].allocations:
            if not isinstance(alloc, mybir.MemoryLocationSet):
                continue
            name = alloc.memorylocations[0].name
            if alloc.kind == "ExternalInput":
                if name != pname:
                    in_names.append(name)
            elif alloc.kind == "ExternalOutput":
                out_names.append(name)
                out_avals.append(jax.core.ShapedArray(
                    tuple(alloc.tensor_shape), mybir.dt.np(alloc.dtype)))
        self.in_names, self.out_names, self.out_avals = in_names, out_names, out_avals
        all_names = in_names + out_names + ([pname] if pname else [])

        def _body(*args):
            operands = list(args)
            if pname is not None:
                operands.append(partition_id_tensor())
            return tuple(_bass_exec_p.bind(
                *operands,
                out_avals=tuple(out_avals),
                in_names=tuple(all_names),
                out_names=tuple(out_names),
                lowering_input_output_aliases=(),
                sim_require_finite=False,
                sim_require_nnan=False,
                nc=nc,
            ))

        devices = jax.devices()[:n_cores]
        self.mesh = Mesh(np.asarray(devices), ("core",))
        self.sharding = NamedSharding(self.mesh, PartitionSpec("core"))
        n_io = len(in_names) + len(out_names)
        self.fn = jax.jit(
            shard_map(_body, mesh=self.mesh,
                      in_specs=(PartitionSpec("core"),) * n_io,
                      out_specs=(PartitionSpec("core"),) * len(out_names),
                      check_rep=False),
            keep_unused=True,
        )
        self.resident = {}   # name -> device-resident jax array
        self.zero_outs = None
        self.xkey = None
        self.wkey = None

    def put(self, name, arr):
        self.resident[name] = jax.device_put(arr, self.sharding)

    def ensure_outs(self):
        if self.zero_outs is None:
            self.zero_outs = [
                jax.device_put(
                    np.zeros((self.n_cores * a.shape[0], *a.shape[1:]), a.dtype),
                    self.sharding)
                for a in self.out_avals]

    def dispatch(self):
        """Launch with all-resident args; returns unfetched device outputs."""
        self.ensure_outs()
        args = [self.resident[n] for n in self.in_names] + self.zero_outs
        return self.fn(*args)


def _build_bass(KA, KB, CA, CB, cumA, cumB):
    """One SPMD kernel: x-allgather, L1 gathers+block, h-allgather,
    L2 gathers+block, decoder. KA/KB: per-tile (group-padded) K."""
    nc = bacc.Bacc("TRN2", target_bir_lowering=False, debug=False,
                   num_devices=NCORES)
    f32, bf16 = mybir.dt.float32, mybir.dt.bfloat16
    i32 = mybir.dt.int32
    Relu = mybir.ActivationFunctionType.Relu
    Sigmoid = mybir.ActivationFunctionType.Sigmoid
    RG = [list(range(NCORES))]

    x_shard = nc.dram_tensor("x_shard", [NSH, CIN], bf16, kind="ExternalInput")
    Ws1 = nc.dram_tensor("Ws1", [3 * CIN + 1, COUT], f32, kind="ExternalInput")
    Ws2 = nc.dram_tensor("Ws2", [3 * COUT + 1, COUT], f32, kind="ExternalInput")
    Wd1a = nc.dram_tensor("Wd1a", [COUT + 1, COUT], f32, kind="ExternalInput")
    Wd2a = nc.dram_tensor("Wd2a", [COUT + 1, 1], f32, kind="ExternalInput")
    idx_a1 = nc.dram_tensor("idx_a1", [128, CA], i32, kind="ExternalInput")
    idx_b1 = nc.dram_tensor("idx_b1", [128, CB], i32, kind="ExternalInput")
    idx_a2 = nc.dram_tensor("idx_a2", [128, CA], i32, kind="ExternalInput")
    idx_b2 = nc.dram_tensor("idx_b2", [128, CB], i32, kind="ExternalInput")
    own_idx = nc.dram_tensor("own_idx", [128, NT], i32, kind="ExternalInput")
    recip6 = nc.dram_tensor("recip6", [128, NT * CIN], f32, kind="ExternalInput")
    recip32 = nc.dram_tensor("recip32", [128, NT * COUT], f32, kind="ExternalInput")
    dec = nc.dram_tensor("dec", [NPC, 1], bf16, kind="ExternalOutput")

    x_table = nc.dram_tensor("x_table", [N_NODES + 8, CIN], bf16,
                             kind="Internal", addr_space="Shared")
    x_stage = nc.dram_tensor("x_stage", [NSH, CIN], bf16, kind="Internal")
    h_own = nc.dram_tensor("h_own", [NPC, COUT], bf16, kind="Internal")
    h_table = nc.dram_tensor("h_table", [NCORES * NPC + 8, COUT], bf16,
                             kind="Internal", addr_space="Shared")

    with TileContext(nc) as tc:
        with tc.tile_pool(name="const", bufs=1) as cpool, \
             tc.tile_pool(name="sbuf", bufs=2) as pool, \
             tc.tile_pool(name="psum", bufs=1, space="PSUM") as psum:
            ident = cpool.tile([128, 128], f32)
            make_identity(nc, ident[:])
            Ws1_sb = cpool.tile([3 * CIN + 1, COUT], f32)
            nc.sync.dma_start(out=Ws1_sb[:], in_=Ws1[:, :])
            Ws2_sb = cpool.tile([3 * COUT + 1, COUT], f32)
            nc.sync.dma_start(out=Ws2_sb[:], in_=Ws2[:, :])
            Wd1_sb = cpool.tile([COUT + 1, COUT], f32)
            nc.sync.dma_start(out=Wd1_sb[:], in_=Wd1a[:, :])
            Wd2_sb = cpool.tile([COUT + 1, 1], f32)
            nc.sync.dma_start(out=Wd2_sb[:], in_=Wd2a[:, :])
            r6_sb = cpool.tile([128, NT * CIN], f32)
            nc.sync.dma_start(out=r6_sb[:], in_=recip6[:, :])
            r32_sb = cpool.tile([128, NT * COUT], f32)
            nc.sync.dma_start(out=r32_sb[:], in_=recip32[:, :])

            # ---- x all-gather + zero pad row ----
            nc.sync.dma_start(out=x_stage[:, :], in_=x_shard[:, :])
            nc.gpsimd.collective_compute(
                "AllGather", mybir.AluOpType.bypass, RG,
                ins=[x_stage[:, :]], outs=[x_table[0:N_NODES, :]])
            zf = cpool.tile([1, CIN], bf16)
            nc.vector.memset(zf[:], 0.0)
            nc.sync.dma_start(out=x_table[XPAD:XPAD + 1, :], in_=zf[:])
            zb = cpool.tile([1, COUT], bf16)
            nc.vector.memset(zb[:], 0.0)
            nc.sync.dma_start(out=h_table[HPAD:HPAD + 1, :], in_=zb[:])

            # ---- own-x gather (gapped) ----
            oidx = cpool.tile([128, NT], i32)
            nc.sync.dma_start(out=oidx[:], in_=own_idx[:, :])
            xo = cpool.tile([128, NT * WA], bf16)
            _emit_gather(nc, cpool, xo, WA, CIN, oidx, NT, x_table)

            h1own = cpool.tile([128, NT * COUT], f32)

            # ================= layer 1 =================
            for G in range(NG):
                g0 = G * GT
                Ka, Kb = int(KA[g0]), int(KB[g0])
                ca0, cb0 = int(cumA[g0]), int(cumB[g0])
                cols_a, cols_b = GT * Ka, GT * Kb

                ia = pool.tile([128, cols_a], i32, tag="ia1")
                nc.sync.dma_start(out=ia[:], in_=idx_a1[:, ca0:ca0 + cols_a])
                ga = pool.tile([128, cols_a * WA], bf16, tag="ga1")
                _emit_gather(nc, pool, ga, WA, CIN, ia, cols_a, x_table)

                ib = pool.tile([128, cols_b], i32, tag="ib1")
                nc.sync.dma_start(out=ib[:], in_=idx_b1[:, cb0:cb0 + cols_b])
                gb = pool.tile([128, cols_b * WA], bf16, tag="gb1")
                _emit_gather(nc, pool, gb, WA, CIN, ib, cols_b, x_table)

                U = pool.tile([128, GT * (3 * CIN + 1)], f32, tag="U1")
                Uv = U[:].rearrange("p (t c) -> p t c", c=3 * CIN + 1)
                nc.vector.tensor_reduce(
                    Uv[:, :, 0:CIN],
                    ga[:].rearrange("p (t k w) -> p t w k", k=Ka, w=WA)[:, :, 0:CIN, :],
                    axis=mybir.AxisListType.X, op=mybir.AluOpType.add)
                tmpb = pool.tile([128, GT * CIN], f32, tag="tb1")
                nc.vector.tensor_reduce(
                    tmpb[:].rearrange("p (t c) -> p t c", c=CIN),
                    gb[:].rearrange("p (t k w) -> p t w k", k=Kb, w=WA)[:, :, 0:CIN, :],
                    axis=mybir.AxisListType.X, op=mybir.AluOpType.add)
                nc.vector.tensor_tensor(
                    out=Uv[:, :, CIN:2 * CIN],
                    in0=tmpb[:].rearrange("p (t c) -> p t c", c=CIN),
                    in1=r6_sb[:, g0 * CIN:(g0 + GT) * CIN].rearrange(
                        "p (t c) -> p t c", c=CIN),
                    op=mybir.AluOpType.mult)
                nc.vector.tensor_copy(
                    Uv[:, :, 2 * CIN:3 * CIN],
                    xo[:].rearrange("p (t w) -> p t w", w=WA)[:, g0:g0 + GT, 0:CIN])
                nc.vector.memset(Uv[:, :, 3 * CIN:3 * CIN + 1], 1.0)

                hbf = pool.tile([128, GT * COUT], bf16, tag="hbf")
                for t in range(GT):
                    j = g0 + t
                    UT_ps = psum.tile([3 * CIN + 1, 128], f32, tag="UT1")
                    nc.tensor.transpose(UT_ps[:], Uv[:, t, :], ident[:])
                    UT = pool.tile([3 * CIN + 1, 128], f32, tag="UT1s")
                    nc.vector.tensor_copy(UT[:], UT_ps[:])
                    h_ps = psum.tile([128, COUT], f32, tag="h1p")
                    nc.tensor.matmul(h_ps[:], lhsT=UT[:], rhs=Ws1_sb[:],
                                     start=True, stop=True)
                    nc.scalar.activation(
                        h1own[:].rearrange("p (t c) -> p t c", c=COUT)[:, j, :],
                        h_ps[:], Relu)
                    nc.vector.tensor_copy(
                        hbf[:].rearrange("p (t c) -> p t c", c=COUT)[:, t, :],
                        h1own[:].rearrange("p (t c) -> p t c", c=COUT)[:, j, :])
                nc.sync.dma_start(
                    out=h_own[g0 * 128:(g0 + GT) * 128, :].rearrange(
                        "(t p) c -> p t c", p=128),
                    in_=hbf[:].rearrange("p (t c) -> p t c", c=COUT))

            # ---- h all-gather ----
            nc.gpsimd.collective_compute(
                "AllGather", mybir.AluOpType.bypass, RG,
                ins=[h_own[:, :]], outs=[h_table[0:NCORES * NPC, :]])

            # ================= layer 2 + decoder =================
            for G in range(NG):
                g0 = G * GT
                Ka, Kb = int(KA[g0]), int(KB[g0])
                ca0, cb0 = int(cumA[g0]), int(cumB[g0])
                cols_a, cols_b = GT * Ka, GT * Kb

                ia = pool.tile([128, cols_a], i32, tag="ia2")
                nc.sync.dma_start(out=ia[:], in_=idx_a2[:, ca0:ca0 + cols_a])
                ga = pool.tile([128, cols_a * WB], bf16, tag="ga2")
                _emit_gather(nc, pool, ga, WB, COUT, ia, cols_a, h_table)

                ib = pool.tile([128, cols_b], i32, tag="ib2")
                nc.sync.dma_start(out=ib[:], in_=idx_b2[:, cb0:cb0 + cols_b])
                gb = pool.tile([128, cols_b * WB], bf16, tag="gb2")
                _emit_gather(nc, pool, gb, WB, COUT, ib, cols_b, h_table)

                U = pool.tile([128, GT * (3 * COUT + 1)], f32, tag="U2")
                Uv = U[:].rearrange("p (t c) -> p t c", c=3 * COUT + 1)
                nc.vector.tensor_reduce(
                    Uv[:, :, 0:COUT],
                    ga[:].rearrange("p (t k w) -> p t w k", k=Ka, w=WB)[:, :, 0:COUT, :],
                    axis=mybir.AxisListType.X, op=mybir.AluOpType.add)
                tmpb = pool.tile([128, GT * COUT], f32, tag="tb2")
                nc.vector.tensor_reduce(
                    tmpb[:].rearrange("p (t c) -> p t c", c=COUT),
                    gb[:].rearrange("p (t k w) -> p t w k", k=Kb, w=WB)[:, :, 0:COUT, :],
                    axis=mybir.AxisListType.X, op=mybir.AluOpType.add)
                nc.vector.tensor_tensor(
                    out=Uv[:, :, COUT:2 * COUT],
                    in0=tmpb[:].rearrange("p (t c) -> p t c", c=COUT),
                    in1=r32_sb[:, g0 * COUT:(g0 + GT) * COUT].rearrange(
                        "p (t c) -> p t c", c=COUT),
                    op=mybir.AluOpType.mult)
                nc.vector.tensor_copy(
                    Uv[:, :, 2 * COUT:3 * COUT],
                    h1own[:].rearrange("p (t c) -> p t c", c=COUT)[:, g0:g0 + GT, :])
                nc.vector.memset(Uv[:, :, 3 * COUT:3 * COUT + 1], 1.0)

                h2T = pool.tile([COUT + 1, GT * 128], f32, tag="h2T")
                nc.vector.memset(
                    h2T[:].rearrange("q (t p) -> q t p", p=128)[COUT:COUT + 1, :, :], 1.0)
                zT = pool.tile([COUT + 1, GT * 128], f32, tag="zT")
                nc.vector.memset(
                    zT[:].rearrange("q (t p) -> q t p", p=128)[COUT:COUT + 1, :, :], 1.0)
                og = pool.tile([128, GT], bf16, tag="og")
                for t in range(GT):
                    UT_ps = psum.tile([3 * COUT + 1, 128], f32, tag="UT2")
                    nc.tensor.transpose(UT_ps[:], Uv[:, t, :], ident[:])
                    UT = pool.tile([3 * COUT + 1, 128], f32, tag="UT2s")
                    nc.vector.tensor_copy(UT[:], UT_ps[:])
                    h2T_ps = psum.tile([COUT, 128], f32, tag="h2Tp")
                    nc.tensor.matmul(h2T_ps[:], lhsT=Ws2_sb[:], rhs=UT[:],
                                     start=True, stop=True)
                    nc.scalar.activation(
                        h2T[:].rearrange("q (t p) -> q t p", p=128)[0:COUT, t, :],
                        h2T_ps[:], Relu)
                    zT_ps = psum.tile([COUT, 128], f32, tag="zTp")
                    nc.tensor.matmul(
                        zT_ps[:], lhsT=Wd1_sb[:],
                        rhs=h2T[:].rearrange("q (t p) -> q t p", p=128)[:, t, :],
                        start=True, stop=True)
                    nc.scalar.activation(
                        zT[:].rearrange("q (t p) -> q t p", p=128)[0:COUT, t, :],
                        zT_ps[:], Relu)
                    o_ps = psum.tile([128, 1], f32, tag="op")
                    nc.tensor.matmul(
                        o_ps[:],
                        lhsT=zT[:].rearrange("q (t p) -> q t p", p=128)[:, t, :],
                        rhs=Wd2_sb[:], start=True, stop=True)
                    nc.scalar.activation(og[:, t:t + 1], o_ps[:], Sigmoid)
                nc.sync.dma_start(
                    out=dec[g0 * 128:(g0 + GT) * 128, :].rearrange(
                        "(t p) c -> p t c", p=128),
                    in_=og[:].rearrange("p (t c) -> p t c", c=1))

    nc.compile()
    return nc


def _prep(edge_tp, edge_int):
    deg_a = np.bincount(edge_tp[1], minlength=N_NODES).astype(np.int64)
    deg_b = np.bincount(edge_int[1], minlength=N_NODES).astype(np.int64)
    order = np.lexsort((deg_b, deg_a))
    rank = np.empty(N_NODES, np.int64)
    rank[order] = np.arange(N_NODES)
    gt = rank // 128                       # global tile of node
    p_of = (rank % 128).astype(np.int64)
    core_of = (gt % NCORES).astype(np.int64)
    j_of = (gt // NCORES).astype(np.int64)
    tpos = core_of * NPC + j_of * 128 + p_of

    NRANK = NCORES * NPC
    dega_r = np.zeros(NRANK, np.int64)
    degb_r = np.zeros(NRANK, np.int64)
    dega_r[rank] = deg_a
    degb_r[rank] = deg_b
    Ka_t = np.maximum(dega_r.reshape(NT, NCORES * 128).max(1), 1)
    Kb_t = np.maximum(degb_r.reshape(NT, NCORES * 128).max(1), 1)
    KA = np.repeat(Ka_t.reshape(NG, GT).max(1), GT)
    KB = np.repeat(Kb_t.reshape(NG, GT).max(1), GT)
    cumA = np.concatenate([[0], np.cumsum(KA)]).astype(np.int64)
    cumB = np.concatenate([[0], np.cumsum(KB)]).astype(np.int64)
    CA, CB = int(cumA[-1]), int(cumB[-1])

    def fill(edges, K, cum, values, pad, width):
        """idx array [NCORES, 128, width]: slot (core,p,col) -> value of src."""
        src = edges[0].astype(np.int64)
        dst = edges[1].astype(np.int64)
        key = tpos[dst]
        o2 = np.argsort(key, kind="stable")
        src_s, key_s = src[o2], key[o2]
        uniq, starts, cnts = np.unique(key_s, return_index=True,
                                       return_counts=True)
        k_idx = np.arange(len(src_s)) - np.repeat(starts, cnts)
        c_s = key_s // NPC
        r_s = key_s % NPC
        j_s = r_s // 128
        p_s = r_s % 128
        col = cum[j_s] + k_idx
        out = np.full((NCORES, 128, width), pad, np.int32)
        out[c_s, p_s, col] = values[src_s]
        return out

    node_id = np.arange(N_NODES)
    ia1 = fill(edge_tp, KA, cumA, node_id, XPAD, CA)
    ib1 = fill(edge_int, KB, cumB, node_id, XPAD, CB)
    ia2 = fill(edge_tp, KA, cumA, tpos, HPAD, CA)
    ib2 = fill(edge_int, KB, cumB, tpos, HPAD, CB)

    own = np.full((NCORES, 128, NT), XPAD, np.int32)
    own[core_of, p_of, j_of] = node_id
    rec = np.ones((NCORES, 128, NT), np.float32)
    rec[core_of, p_of, j_of] = 1.0 / np.maximum(deg_b, 1.0)
    recip6 = np.ascontiguousarray(
        np.broadcast_to(rec[:, :, :, None], (NCORES, 128, NT, CIN))
    ).reshape(NCORES, 128, NT * CIN).astype(np.float32)
    recip32 = np.ascontiguousarray(
        np.broadcast_to(rec[:, :, :, None], (NCORES, 128, NT, COUT))
    ).reshape(NCORES, 128, NT * COUT).astype(np.float32)

    unperm = np.empty(N_NODES, np.int64)   # out[node] = dec_flat[unperm[node]]
    unperm[order] = tpos[order]
    return (KA, KB, CA, CB, cumA, cumB, ia1, ib1, ia2, ib2, own,
            recip6, recip32, unperm)


def _stack_weights(W_self1, b1, W_tp1, W_int1, W_res1,
                   W_self2, b2, W_tp2, W_int2, Wd1, bd1, Wd2, bd2):
    Ws1 = np.zeros((3 * CIN + 1, COUT), np.float32)
    Ws1[0:CIN] = np.asarray(W_tp1)
    Ws1[CIN:2 * CIN] = np.asarray(W_int1)
    Ws1[2 * CIN:3 * CIN] = np.asarray(W_self1) + np.asarray(W_res1)
    Ws1[3 * CIN] = np.asarray(b1)
    Ws2 = np.zeros((3 * COUT + 1, COUT), np.float32)
    Ws2[0:COUT] = np.asarray(W_tp2)
    Ws2[COUT:2 * COUT] = np.asarray(W_int2)
    Ws2[2 * COUT:3 * COUT] = np.asarray(W_self2) + np.eye(COUT, dtype=np.float32)
    Ws2[3 * COUT] = np.asarray(b2)
    Wd1_a = np.zeros((COUT + 1, COUT), np.float32)
    Wd1_a[0:COUT] = np.asarray(Wd1)
    Wd1_a[COUT] = np.asarray(bd1)
    Wd2_a = np.zeros((COUT + 1, 1), np.float32)
    Wd2_a[0:COUT] = np.asarray(Wd2).reshape(COUT, 1)
    Wd2_a[COUT] = np.asarray(bd2).ravel()[0]
    return Ws1, Ws2, Wd1_a, Wd2_a


def kernel(x, edge_tp, edge_int,
           W_self1, b1, W_tp1, W_int1, W_res1,
           W_self2, b2, W_tp2, W_int2,
           Wd1, bd1, Wd2, bd2):
    x = np.ascontiguousarray(np.asarray(x, np.float32))
    edge_tp = np.asarray(edge_tp)
    edge_int = np.asarray(edge_int)
    key = hashlib.sha1(edge_tp[:, ::997].tobytes()
                       + edge_int[:, ::997].tobytes()).hexdigest()
    if key not in _CACHE:
        prep = _prep(edge_tp, edge_int)
        (KA, KB, CA, CB, cumA, cumB, ia1, ib1, ia2, ib2, own,
         recip6, recip32, unperm) = prep
        nc = _build_bass(KA, KB, CA, CB, cumA, cumB)
        ck = _Compiled(nc, NCORES)
        ck.put("idx_a1", ia1.reshape(NCORES * 128, CA))
        ck.put("idx_b1", ib1.reshape(NCORES * 128, CB))
        ck.put("idx_a2", ia2.reshape(NCORES * 128, CA))
        ck.put("idx_b2", ib2.reshape(NCORES * 128, CB))
        ck.put("own_idx", own.reshape(NCORES * 128, NT))
        ck.put("recip6", recip6.reshape(NCORES * 128, NT * CIN))
        ck.put("recip32", recip32.reshape(NCORES * 128, NT * COUT))
        _CACHE[key] = (unperm, ck)
    unperm, ck = _CACHE[key]

    # --- x: content-hashed resident upload (async on miss) ---
    xkey = hashlib.sha1(x).digest()
    if ck.xkey != xkey:
        xb = np.ascontiguousarray(x.astype(BF16)).reshape(NCORES * NSH, CIN)
        ck.put("x_shard", xb)
        ck.xkey = xkey

    # --- weights: content-hashed resident upload (async on miss) ---
    wparts = [np.ascontiguousarray(np.asarray(w, np.float32)) for w in (
        W_self1, b1, W_tp1, W_int1, W_res1, W_self2, b2, W_tp2, W_int2,
        Wd1, bd1, Wd2, bd2)]
    h = hashlib.sha1()
    for w in wparts:
        h.update(w)
    wkey = h.digest()
    if ck.wkey != wkey:
        Ws1, Ws2, Wd1_a, Wd2_a = _stack_weights(*wparts)
        ck.put("Ws1", np.tile(Ws1, (NCORES, 1)))
        ck.put("Ws2", np.tile(Ws2, (NCORES, 1)))
        ck.put("Wd1a", np.tile(Wd1_a, (NCORES, 1)))
        ck.put("Wd2a", np.tile(Wd2_a, (NCORES, 1)))
        ck.wkey = wkey

    # --- single-sync pipeline: (put) -> exec -> fetch ---
    outs = ck.dispatch()
    dec = np.asarray(outs[0]).reshape(NCORES * NPC)
    return dec[unperm].reshape(N_NODES, 1).astype(np.float32)


# revision 13
# speedup vs baseline: 1.1914x; 1.1914x over previous
"""Trainium2 Bass kernel for 2-layer hetero GNN + MLP decoder — device-gather design.

Single NEFF, single launch per call:
  - nodes dealt to 8 cores by degree-sorted global tiles (tile g -> core g%8),
    shared padded-K slot schedule per 14-tile group;
  - device AllGather replicates x (and later h1) into per-core DRAM tables;
  - per-edge source rows fetched with indirect (dynamic-descriptor) DMA
    gathers into a GAPPED SBUF layout (one descriptor per slot);
  - DVE grouped segmented reduce -> per-tile PE transpose + matmul
    (weights stacked [tp | int | self+res | bias]);
  - layer2 + decoder fused via reversed matmuls (h2T = W^T @ UT2).
Host per call: bf16-cast x, content-hash inputs to skip re-uploads,
single-sync async pipeline (put -> exec -> fetch), unpermute [N,1] output.
"""
import sys
import hashlib

sys.path.insert(0, '/opt/trn_rl_repo')

import numpy as np
import ml_dtypes

import jax
from jax.sharding import Mesh, PartitionSpec, NamedSharding
from jax.experimental.shard_map import shard_map

import concourse.bass as bass
import concourse.bacc as bacc
import concourse.mybir as mybir
from concourse.tile import TileContext
from concourse.masks import make_identity
from concourse.bass2jax import _bass_exec_p, partition_id_tensor, install_neuronx_cc_hook

N_NODES = 100000
N_EDGES = 1600000
CIN, COUT = 6, 32
NCORES = 8
NT = 98                     # slot tiles per core
NPC = NT * 128              # padded nodes per core (12544)
NSH = N_NODES // NCORES     # x shard rows (12500)
GT = 7                      # tiles per group
NG = NT // GT               # 14 groups
WA = CIN                    # slot width, layer1 (bf16, gapless)
WB = COUT                   # slot width, layer2 (bf16, gapless)
XPAD = N_NODES              # zero row index in x table
HPAD = NCORES * NPC         # zero row index in h table (100352)
BF16 = ml_dtypes.bfloat16

_CACHE = {}


def _emit_gather(nc, pool, out_tile, w, c_use, idx_sb, ncols, table):
    """Gather rows of `table` into out_tile [128, ncols*w] using
    idx_sb [128, ncols]. One indirect DMA per column (the SWDGE ucode
    only pairs one offset per partition per instruction)."""
    v = out_tile[:].rearrange("p (s w) -> p s w", w=w)
    for k in range(ncols):
        nc.gpsimd.indirect_dma_start(
            out=v[:, k, 0:c_use],
            out_offset=None, in_=table[:, :],
            in_offset=bass.IndirectOffsetOnAxis(ap=idx_sb[:, k:k + 1], axis=0))


class _Compiled:
    """Compile-once PJRT executor for one Bass module on 8 cores."""

    def __init__(self, nc, n_cores):
        install_neuronx_cc_hook()
        self.nc = nc
        self.n_cores = n_cores
        pname = nc.partition_id_tensor.name if nc.partition_id_tensor else None
        in_names, out_names, out_avals = [], [], []
        for alloc in nc.m.functions[0].allocations:
            if not isinstance(alloc, mybir.MemoryLocationSet):
                continue
            name = alloc.memorylocations[0].name
            if alloc.kind == "ExternalInput":
                if name != pname:
                    in_names.append(name)
            elif alloc.kind == "ExternalOutput":
                out_names.append(name)
                out_avals.append(jax.core.ShapedArray(
                    tuple(alloc.tensor_shape), mybir.dt.np(alloc.dtype)))
        self.in_names, self.out_names, self.out_avals = in_names, out_names, out_avals
        all_names = in_names + out_names + ([pname] if pname else [])

        def _body(*args):
            operands = list(args)
            if pname is not None:
                operands.append(partition_id_tensor())
            return tuple(_bass_exec_p.bind(
                *operands,
                out_avals=tuple(out_avals),
                in_names=tuple(all_names),
                out_names=tuple(out_names),
                lowering_input_output_aliases=(),
                sim_require_finite=False,
                sim_require_nnan=False,
                nc=nc,
            ))

        devices = jax.devices()[:n_cores]
        self.mesh = Mesh(np.asarray(devices), ("core",))
        self.sharding = NamedSharding(self.mesh, PartitionSpec("core"))
        n_io = len(in_names) + len(out_names)
        self.fn = jax.jit(
            shard_map(_body, mesh=self.mesh,
                      in_specs=(PartitionSpec("core"),) * n_io,
                      out_specs=(PartitionSpec("core"),) * len(out_names),
                      check_rep=False),
            keep_unused=True,
        )
        self.resident = {}   # name -> device-resident jax array
        self.zero_outs = None
        self.xkey = None
        self.wkey = None

    def put(self, name, arr):
        self.resident[name] = jax.device_put(arr, self.sharding)

    def ensure_outs(self):
        if self.zero_outs is None:
            self.zero_outs = [
                jax.device_put(
                    np.zeros((self.n_cores * a.shape[0], *a.shape[1:]), a.dtype),
                    self.sharding)
                for a in self.out_avals]

    def dispatch(self):
        """Launch with all-resident args; returns unfetched device outputs."""
        self.ensure_outs()
        args = [self.resident[n] for n in self.in_names] + self.zero_outs
        return self.fn(*args)


def _build_bass(KA, KB, CA, CB, cumA, cumB):
    """One SPMD kernel: x-allgather, L1 gathers+block, h-allgather,
    L2 gathers+block, decoder. KA/KB: per-tile (group-padded) K."""
    nc = bacc.Bacc("TRN2", target_bir_lowering=False, debug=False,
                   num_devices=NCORES)
    f32, bf16 = mybir.dt.float32, mybir.dt.bfloat16
    i32 = mybir.dt.int32
    Relu = mybir.ActivationFunctionType.Relu
    Sigmoid = mybir.ActivationFunctionType.Sigmoid
    RG = [list(range(NCORES))]

    x_shard = nc.dram_tensor("x_shard", [NSH, CIN], bf16, kind="ExternalInput")
    Ws1 = nc.dram_tensor("Ws1", [3 * CIN + 1, COUT], f32, kind="ExternalInput")
    Ws2 = nc.dram_tensor("Ws2", [3 * COUT + 1, COUT], f32, kind="ExternalInput")
    Wd1a = nc.dram_tensor("Wd1a", [COUT + 1, COUT], f32, kind="ExternalInput")
    Wd2a = nc.dram_tensor("Wd2a", [COUT + 1, 1], f32, kind="ExternalInput")
    idx_a1 = nc.dram_tensor("idx_a1", [128, CA], i32, kind="ExternalInput")
    idx_b1 = nc.dram_tensor("idx_b1", [128, CB], i32, kind="ExternalInput")
    idx_a2 = nc.dram_tensor("idx_a2", [128, CA], i32, kind="ExternalInput")
    idx_b2 = nc.dram_tensor("idx_b2", [128, CB], i32, kind="ExternalInput")
    own_idx = nc.dram_tensor("own_idx", [128, NT], i32, kind="ExternalInput")
    recip6 = nc.dram_tensor("recip6", [128, NT * CIN], f32, kind="ExternalInput")
    recip32 = nc.dram_tensor("recip32", [128, NT * COUT], f32, kind="ExternalInput")
    dec = nc.dram_tensor("dec", [NPC, 1], bf16, kind="ExternalOutput")

    x_table = nc.dram_tensor("x_table", [N_NODES + 8, CIN], bf16,
                             kind="Internal", addr_space="Shared")
    x_stage = nc.dram_tensor("x_stage", [NSH, CIN], bf16, kind="Internal")
    h_own = nc.dram_tensor("h_own", [NPC, COUT], bf16, kind="Internal")
    h_table = nc.dram_tensor("h_table", [NCORES * NPC + 8, COUT], bf16,
                             kind="Internal", addr_space="Shared")

    with TileContext(nc) as tc:
        with tc.tile_pool(name="const", bufs=1) as cpool, \
             tc.tile_pool(name="sbuf", bufs=2) as pool, \
             tc.tile_pool(name="psum", bufs=1, space="PSUM") as psum:
            ident = cpool.tile([128, 128], f32)
            make_identity(nc, ident[:])
            Ws1_sb = cpool.tile([3 * CIN + 1, COUT], f32)
            nc.sync.dma_start(out=Ws1_sb[:], in_=Ws1[:, :])
            Ws2_sb = cpool.tile([3 * COUT + 1, COUT], f32)
            nc.sync.dma_start(out=Ws2_sb[:], in_=Ws2[:, :])
            Wd1_sb = cpool.tile([COUT + 1, COUT], f32)
            nc.sync.dma_start(out=Wd1_sb[:], in_=Wd1a[:, :])
            Wd2_sb = cpool.tile([COUT + 1, 1], f32)
            nc.sync.dma_start(out=Wd2_sb[:], in_=Wd2a[:, :])
            r6_sb = cpool.tile([128, NT * CIN], f32)
            nc.sync.dma_start(out=r6_sb[:], in_=recip6[:, :])
            r32_sb = cpool.tile([128, NT * COUT], f32)
            nc.sync.dma_start(out=r32_sb[:], in_=recip32[:, :])

            # ---- x all-gather + zero pad row ----
            nc.sync.dma_start(out=x_stage[:, :], in_=x_shard[:, :])
            nc.gpsimd.collective_compute(
                "AllGather", mybir.AluOpType.bypass, RG,
                ins=[x_stage[:, :]], outs=[x_table[0:N_NODES, :]])
            zf = cpool.tile([1, CIN], bf16)
            nc.vector.memset(zf[:], 0.0)
            nc.sync.dma_start(out=x_table[XPAD:XPAD + 1, :], in_=zf[:])
            zb = cpool.tile([1, COUT], bf16)
            nc.vector.memset(zb[:], 0.0)
            nc.sync.dma_start(out=h_table[HPAD:HPAD + 1, :], in_=zb[:])

            # ---- own-x gather (gapped) ----
            oidx = cpool.tile([128, NT], i32)
            nc.sync.dma_start(out=oidx[:], in_=own_idx[:, :])
            xo = cpool.tile([128, NT * WA], bf16)
            _emit_gather(nc, cpool, xo, WA, CIN, oidx, NT, x_table)

            h1own = cpool.tile([128, NT * COUT], f32)

            # ================= layer 1 =================
            for G in range(NG):
                g0 = G * GT
                ca0, cb0 = int(cumA[g0]), int(cumB[g0])
                cols_a = int(cumA[g0 + GT] - ca0)
                cols_b = int(cumB[g0 + GT] - cb0)

                ia = pool.tile([128, cols_a], i32, tag="ia1")
                nc.sync.dma_start(out=ia[:], in_=idx_a1[:, ca0:ca0 + cols_a])
                ga = pool.tile([128, cols_a * WA], bf16, tag="ga1")
                _emit_gather(nc, pool, ga, WA, CIN, ia, cols_a, x_table)

                ib = pool.tile([128, cols_b], i32, tag="ib1")
                nc.sync.dma_start(out=ib[:], in_=idx_b1[:, cb0:cb0 + cols_b])
                gb = pool.tile([128, cols_b * WA], bf16, tag="gb1")
                _emit_gather(nc, pool, gb, WA, CIN, ib, cols_b, x_table)

                U = pool.tile([128, GT * (3 * CIN + 1)], f32, tag="U1")
                Uv = U[:].rearrange("p (t c) -> p t c", c=3 * CIN + 1)
                tmpb = pool.tile([128, GT * CIN], f32, tag="tb1")
                for t in range(GT):
                    j = g0 + t
                    oa, ka = int(cumA[j] - ca0), int(KA[j])
                    nc.vector.tensor_reduce(
                        Uv[:, t, 0:CIN],
                        ga[:, oa * WA:(oa + ka) * WA].rearrange(
                            "p (k w) -> p w k", w=WA),
                        axis=mybir.AxisListType.X, op=mybir.AluOpType.add)
                    ob, kb = int(cumB[j] - cb0), int(KB[j])
                    nc.vector.tensor_reduce(
                        tmpb[:, t * CIN:(t + 1) * CIN],
                        gb[:, ob * WA:(ob + kb) * WA].rearrange(
                            "p (k w) -> p w k", w=WA),
                        axis=mybir.AxisListType.X, op=mybir.AluOpType.add)
                nc.vector.tensor_tensor(
                    out=Uv[:, :, CIN:2 * CIN],
                    in0=tmpb[:].rearrange("p (t c) -> p t c", c=CIN),
                    in1=r6_sb[:, g0 * CIN:(g0 + GT) * CIN].rearrange(
                        "p (t c) -> p t c", c=CIN),
                    op=mybir.AluOpType.mult)
                nc.vector.tensor_copy(
                    Uv[:, :, 2 * CIN:3 * CIN],
                    xo[:].rearrange("p (t w) -> p t w", w=WA)[:, g0:g0 + GT, 0:CIN])
                nc.vector.memset(Uv[:, :, 3 * CIN:3 * CIN + 1], 1.0)

                hbf = pool.tile([128, GT * COUT], bf16, tag="hbf")
                for t in range(GT):
                    j = g0 + t
                    UT_ps = psum.tile([3 * CIN + 1, 128], f32, tag="UT1")
                    nc.tensor.transpose(UT_ps[:], Uv[:, t, :], ident[:])
                    UT = pool.tile([3 * CIN + 1, 128], f32, tag="UT1s")
                    nc.vector.tensor_copy(UT[:], UT_ps[:])
                    h_ps = psum.tile([128, COUT], f32, tag="h1p")
                    nc.tensor.matmul(h_ps[:], lhsT=UT[:], rhs=Ws1_sb[:],
                                     start=True, stop=True)
                    nc.scalar.activation(
                        h1own[:].rearrange("p (t c) -> p t c", c=COUT)[:, j, :],
                        h_ps[:], Relu)
                    nc.vector.tensor_copy(
                        hbf[:].rearrange("p (t c) -> p t c", c=COUT)[:, t, :],
                        h1own[:].rearrange("p (t c) -> p t c", c=COUT)[:, j, :])
                nc.sync.dma_start(
                    out=h_own[g0 * 128:(g0 + GT) * 128, :].rearrange(
                        "(t p) c -> p t c", p=128),
                    in_=hbf[:].rearrange("p (t c) -> p t c", c=COUT))

            # ---- h all-gather ----
            nc.gpsimd.collective_compute(
                "AllGather", mybir.AluOpType.bypass, RG,
                ins=[h_own[:, :]], outs=[h_table[0:NCORES * NPC, :]])

            # ================= layer 2 + decoder =================
            for G in range(NG):
                g0 = G * GT
                ca0, cb0 = int(cumA[g0]), int(cumB[g0])
                cols_a = int(cumA[g0 + GT] - ca0)
                cols_b = int(cumB[g0 + GT] - cb0)

                ia = pool.tile([128, cols_a], i32, tag="ia2")
                nc.sync.dma_start(out=ia[:], in_=idx_a2[:, ca0:ca0 + cols_a])
                ga = pool.tile([128, cols_a * WB], bf16, tag="ga2")
                _emit_gather(nc, pool, ga, WB, COUT, ia, cols_a, h_table)

                ib = pool.tile([128, cols_b], i32, tag="ib2")
                nc.sync.dma_start(out=ib[:], in_=idx_b2[:, cb0:cb0 + cols_b])
                gb = pool.tile([128, cols_b * WB], bf16, tag="gb2")
                _emit_gather(nc, pool, gb, WB, COUT, ib, cols_b, h_table)

                U = pool.tile([128, GT * (3 * COUT + 1)], f32, tag="U2")
                Uv = U[:].rearrange("p (t c) -> p t c", c=3 * COUT + 1)
                tmpb = pool.tile([128, GT * COUT], f32, tag="tb2")
                for t in range(GT):
                    j = g0 + t
                    oa, ka = int(cumA[j] - ca0), int(KA[j])
                    nc.vector.tensor_reduce(
                        Uv[:, t, 0:COUT],
                        ga[:, oa * WB:(oa + ka) * WB].rearrange(
                            "p (k w) -> p w k", w=WB),
                        axis=mybir.AxisListType.X, op=mybir.AluOpType.add)
                    ob, kb = int(cumB[j] - cb0), int(KB[j])
                    nc.vector.tensor_reduce(
                        tmpb[:, t * COUT:(t + 1) * COUT],
                        gb[:, ob * WB:(ob + kb) * WB].rearrange(
                            "p (k w) -> p w k", w=WB),
                        axis=mybir.AxisListType.X, op=mybir.AluOpType.add)
                nc.vector.tensor_tensor(
                    out=Uv[:, :, COUT:2 * COUT],
                    in0=tmpb[:].rearrange("p (t c) -> p t c", c=COUT),
                    in1=r32_sb[:, g0 * COUT:(g0 + GT) * COUT].rearrange(
                        "p (t c) -> p t c", c=COUT),
                    op=mybir.AluOpType.mult)
                nc.vector.tensor_copy(
                    Uv[:, :, 2 * COUT:3 * COUT],
                    h1own[:].rearrange("p (t c) -> p t c", c=COUT)[:, g0:g0 + GT, :])
                nc.vector.memset(Uv[:, :, 3 * COUT:3 * COUT + 1], 1.0)

                h2T = pool.tile([COUT + 1, GT * 128], f32, tag="h2T")
                nc.vector.memset(
                    h2T[:].rearrange("q (t p) -> q t p", p=128)[COUT:COUT + 1, :, :], 1.0)
                zT = pool.tile([COUT + 1, GT * 128], f32, tag="zT")
                nc.vector.memset(
                    zT[:].rearrange("q (t p) -> q t p", p=128)[COUT:COUT + 1, :, :], 1.0)
                og = pool.tile([128, GT], bf16, tag="og")
                for t in range(GT):
                    UT_ps = psum.tile([3 * COUT + 1, 128], f32, tag="UT2")
                    nc.tensor.transpose(UT_ps[:], Uv[:, t, :], ident[:])
                    UT = pool.tile([3 * COUT + 1, 128], f32, tag="UT2s")
                    nc.vector.tensor_copy(UT[:], UT_ps[:])
                    h2T_ps = psum.tile([COUT, 128], f32, tag="h2Tp")
                    nc.tensor.matmul(h2T_ps[:], lhsT=Ws2_sb[:], rhs=UT[:],
                                     start=True, stop=True)
                    nc.scalar.activation(
                        h2T[:].rearrange("q (t p) -> q t p", p=128)[0:COUT, t, :],
                        h2T_ps[:], Relu)
                    zT_ps = psum.tile([COUT, 128], f32, tag="zTp")
                    nc.tensor.matmul(
                        zT_ps[:], lhsT=Wd1_sb[:],
                        rhs=h2T[:].rearrange("q (t p) -> q t p", p=128)[:, t, :],
                        start=True, stop=True)
                    nc.scalar.activation(
                        zT[:].rearrange("q (t p) -> q t p", p=128)[0:COUT, t, :],
                        zT_ps[:], Relu)
                    o_ps = psum.tile([128, 1], f32, tag="op")
                    nc.tensor.matmul(
                        o_ps[:],
                        lhsT=zT[:].rearrange("q (t p) -> q t p", p=128)[:, t, :],
                        rhs=Wd2_sb[:], start=True, stop=True)
                    nc.scalar.activation(og[:, t:t + 1], o_ps[:], Sigmoid)
                nc.sync.dma_start(
                    out=dec[g0 * 128:(g0 + GT) * 128, :].rearrange(
                        "(t p) c -> p t c", p=128),
                    in_=og[:].rearrange("p (t c) -> p t c", c=1))

    nc.compile()
    return nc


def _prep(edge_tp, edge_int):
    deg_a = np.bincount(edge_tp[1], minlength=N_NODES).astype(np.int64)
    deg_b = np.bincount(edge_int[1], minlength=N_NODES).astype(np.int64)
    order = np.lexsort((deg_b, deg_a))
    rank = np.empty(N_NODES, np.int64)
    rank[order] = np.arange(N_NODES)
    gt = rank // 128                       # global tile of node
    p_of = (rank % 128).astype(np.int64)
    core_of = (gt % NCORES).astype(np.int64)
    j_of = (gt // NCORES).astype(np.int64)
    tpos = core_of * NPC + j_of * 128 + p_of

    NRANK = NCORES * NPC
    dega_r = np.zeros(NRANK, np.int64)
    degb_r = np.zeros(NRANK, np.int64)
    dega_r[rank] = deg_a
    degb_r[rank] = deg_b
    # per-tile K (no group padding): each tile row j pads only to its own max
    KA = np.maximum(dega_r.reshape(NT, NCORES * 128).max(1), 1).astype(np.int64)
    KB = np.maximum(degb_r.reshape(NT, NCORES * 128).max(1), 1).astype(np.int64)
    cumA = np.concatenate([[0], np.cumsum(KA)]).astype(np.int64)
    cumB = np.concatenate([[0], np.cumsum(KB)]).astype(np.int64)
    CA, CB = int(cumA[-1]), int(cumB[-1])

    def fill(edges, K, cum, values, pad, width):
        """idx array [NCORES, 128, width]: slot (core,p,col) -> value of src."""
        src = edges[0].astype(np.int64)
        dst = edges[1].astype(np.int64)
        key = tpos[dst]
        o2 = np.argsort(key, kind="stable")
        src_s, key_s = src[o2], key[o2]
        uniq, starts, cnts = np.unique(key_s, return_index=True,
                                       return_counts=True)
        k_idx = np.arange(len(src_s)) - np.repeat(starts, cnts)
        c_s = key_s // NPC
        r_s = key_s % NPC
        j_s = r_s // 128
        p_s = r_s % 128
        col = cum[j_s] + k_idx
        out = np.full((NCORES, 128, width), pad, np.int32)
        out[c_s, p_s, col] = values[src_s]
        return out

    node_id = np.arange(N_NODES)
    ia1 = fill(edge_tp, KA, cumA, node_id, XPAD, CA)
    ib1 = fill(edge_int, KB, cumB, node_id, XPAD, CB)
    ia2 = fill(edge_tp, KA, cumA, tpos, HPAD, CA)
    ib2 = fill(edge_int, KB, cumB, tpos, HPAD, CB)

    own = np.full((NCORES, 128, NT), XPAD, np.int32)
    own[core_of, p_of, j_of] = node_id
    rec = np.ones((NCORES, 128, NT), np.float32)
    rec[core_of, p_of, j_of] = 1.0 / np.maximum(deg_b, 1.0)
    recip6 = np.ascontiguousarray(
        np.broadcast_to(rec[:, :, :, None], (NCORES, 128, NT, CIN))
    ).reshape(NCORES, 128, NT * CIN).astype(np.float32)
    recip32 = np.ascontiguousarray(
        np.broadcast_to(rec[:, :, :, None], (NCORES, 128, NT, COUT))
    ).reshape(NCORES, 128, NT * COUT).astype(np.float32)

    unperm = np.empty(N_NODES, np.int64)   # out[node] = dec_flat[unperm[node]]
    unperm[order] = tpos[order]
    return (KA, KB, CA, CB, cumA, cumB, ia1, ib1, ia2, ib2, own,
            recip6, recip32, unperm)


def _stack_weights(W_self1, b1, W_tp1, W_int1, W_res1,
                   W_self2, b2, W_tp2, W_int2, Wd1, bd1, Wd2, bd2):
    Ws1 = np.zeros((3 * CIN + 1, COUT), np.float32)
    Ws1[0:CIN] = np.asarray(W_tp1)
    Ws1[CIN:2 * CIN] = np.asarray(W_int1)
    Ws1[2 * CIN:3 * CIN] = np.asarray(W_self1) + np.asarray(W_res1)
    Ws1[3 * CIN] = np.asarray(b1)
    Ws2 = np.zeros((3 * COUT + 1, COUT), np.float32)
    Ws2[0:COUT] = np.asarray(W_tp2)
    Ws2[COUT:2 * COUT] = np.asarray(W_int2)
    Ws2[2 * COUT:3 * COUT] = np.asarray(W_self2) + np.eye(COUT, dtype=np.float32)
    Ws2[3 * COUT] = np.asarray(b2)
    Wd1_a = np.zeros((COUT + 1, COUT), np.float32)
    Wd1_a[0:COUT] = np.asarray(Wd1)
    Wd1_a[COUT] = np.asarray(bd1)
    Wd2_a = np.zeros((COUT + 1, 1), np.float32)
    Wd2_a[0:COUT] = np.asarray(Wd2).reshape(COUT, 1)
    Wd2_a[COUT] = np.asarray(bd2).ravel()[0]
    return Ws1, Ws2, Wd1_a, Wd2_a


def kernel(x, edge_tp, edge_int,
           W_self1, b1, W_tp1, W_int1, W_res1,
           W_self2, b2, W_tp2, W_int2,
           Wd1, bd1, Wd2, bd2):
    x = np.ascontiguousarray(np.asarray(x, np.float32))
    edge_tp = np.asarray(edge_tp)
    edge_int = np.asarray(edge_int)
    key = hashlib.sha1(edge_tp[:, ::997].tobytes()
                       + edge_int[:, ::997].tobytes()).hexdigest()
    if key not in _CACHE:
        prep = _prep(edge_tp, edge_int)
        (KA, KB, CA, CB, cumA, cumB, ia1, ib1, ia2, ib2, own,
         recip6, recip32, unperm) = prep
        nc = _build_bass(KA, KB, CA, CB, cumA, cumB)
        ck = _Compiled(nc, NCORES)
        ck.put("idx_a1", ia1.reshape(NCORES * 128, CA))
        ck.put("idx_b1", ib1.reshape(NCORES * 128, CB))
        ck.put("idx_a2", ia2.reshape(NCORES * 128, CA))
        ck.put("idx_b2", ib2.reshape(NCORES * 128, CB))
        ck.put("own_idx", own.reshape(NCORES * 128, NT))
        ck.put("recip6", recip6.reshape(NCORES * 128, NT * CIN))
        ck.put("recip32", recip32.reshape(NCORES * 128, NT * COUT))
        _CACHE[key] = (unperm, ck)
    unperm, ck = _CACHE[key]

    # --- x: content-hashed resident upload (async on miss) ---
    xkey = hashlib.sha1(x).digest()
    if ck.xkey != xkey:
        xb = np.ascontiguousarray(x.astype(BF16)).reshape(NCORES * NSH, CIN)
        ck.put("x_shard", xb)
        ck.xkey = xkey

    # --- weights: content-hashed resident upload (async on miss) ---
    wparts = [np.ascontiguousarray(np.asarray(w, np.float32)) for w in (
        W_self1, b1, W_tp1, W_int1, W_res1, W_self2, b2, W_tp2, W_int2,
        Wd1, bd1, Wd2, bd2)]
    h = hashlib.sha1()
    for w in wparts:
        h.update(w)
    wkey = h.digest()
    if ck.wkey != wkey:
        Ws1, Ws2, Wd1_a, Wd2_a = _stack_weights(*wparts)
        ck.put("Ws1", np.tile(Ws1, (NCORES, 1)))
        ck.put("Ws2", np.tile(Ws2, (NCORES, 1)))
        ck.put("Wd1a", np.tile(Wd1_a, (NCORES, 1)))
        ck.put("Wd2a", np.tile(Wd2_a, (NCORES, 1)))
        ck.wkey = wkey

    # --- single-sync pipeline: (put) -> exec -> fetch ---
    outs = ck.dispatch()
    dec = np.asarray(outs[0]).reshape(NCORES * NPC)
    return dec[unperm].reshape(N_NODES, 1).astype(np.float32)


# revision 15
# speedup vs baseline: 1.2504x; 1.0495x over previous
"""Trainium2 Bass kernel for 2-layer hetero GNN + MLP decoder — device-gather design.

Single NEFF, single launch per call:
  - nodes dealt to 8 cores by degree-sorted global tiles (tile g -> core g%8),
    shared padded-K slot schedule per 14-tile group;
  - device AllGather replicates x (and later h1) into per-core DRAM tables;
  - per-edge source rows fetched with indirect (dynamic-descriptor) DMA
    gathers into a GAPPED SBUF layout (one descriptor per slot);
  - DVE grouped segmented reduce -> per-tile PE transpose + matmul
    (weights stacked [tp | int | self+res | bias]);
  - layer2 + decoder fused via reversed matmuls (h2T = W^T @ UT2).
Host per call: bf16-cast x, content-hash inputs to skip re-uploads,
single-sync async pipeline (put -> exec -> fetch), unpermute [N,1] output.
"""
import sys
import hashlib

sys.path.insert(0, '/opt/trn_rl_repo')

import numpy as np
import ml_dtypes

import jax
from jax.sharding import Mesh, PartitionSpec, NamedSharding
from jax.experimental.shard_map import shard_map

import concourse.bass as bass
import concourse.bacc as bacc
import concourse.mybir as mybir
from concourse.tile import TileContext
from concourse.masks import make_identity
from concourse.bass2jax import _bass_exec_p, partition_id_tensor, install_neuronx_cc_hook

N_NODES = 100000
N_EDGES = 1600000
CIN, COUT = 6, 32
NCORES = 8
NT = 98                     # slot tiles per core
NPC = NT * 128              # padded nodes per core (12544)
NSH = N_NODES // NCORES     # x shard rows (12500)
GT = 7                      # tiles per group
NG = NT // GT               # 14 groups
WA = CIN                    # slot width, layer1 (bf16, gapless)
WB = COUT                   # slot width, layer2 (bf16, gapless)
XPAD = N_NODES              # zero row index in x table
HPAD = NCORES * NPC         # zero row index in h table (100352)
BF16 = ml_dtypes.bfloat16

_CACHE = {}
N_SWDGE_Q = 4   # spread indirect gathers across the 4 SWDGE queues


def _emit_gather(nc, pool, out_tile, w, c_use, idx_sb, ncols, table):
    """Gather rows of `table` into out_tile [128, ncols*w] using
    idx_sb [128, ncols]. One indirect DMA per column (the SWDGE ucode
    only pairs one offset per partition per instruction); columns are
    round-robined over the SWDGE queues."""
    v = out_tile[:].rearrange("p (s w) -> p s w", w=w)
    for k in range(ncols):
        inst = nc.gpsimd.indirect_dma_start(
            out=v[:, k, 0:c_use],
            out_offset=None, in_=table[:, :],
            in_offset=bass.IndirectOffsetOnAxis(ap=idx_sb[:, k:k + 1], axis=0))
        q = k % N_SWDGE_Q
        if q:
            inst.ins.queue = f"qPoolDynamic{q}"


class _Compiled:
    """Compile-once PJRT executor for one Bass module on 8 cores."""

    def __init__(self, nc, n_cores):
        install_neuronx_cc_hook()
        self.nc = nc
        self.n_cores = n_cores
        pname = nc.partition_id_tensor.name if nc.partition_id_tensor else None
        in_names, out_names, out_avals = [], [], []
        for alloc in nc.m.functions[0].allocations:
            if not isinstance(alloc, mybir.MemoryLocationSet):
                continue
            name = alloc.memorylocations[0].name
            if alloc.kind == "ExternalInput":
                if name != pname:
                    in_names.append(name)
            elif alloc.kind == "ExternalOutput":
                out_names.append(name)
                out_avals.append(jax.core.ShapedArray(
                    tuple(alloc.tensor_shape), mybir.dt.np(alloc.dtype)))
        self.in_names, self.out_names, self.out_avals = in_names, out_names, out_avals
        all_names = in_names + out_names + ([pname] if pname else [])

        def _body(*args):
            operands = list(args)
            if pname is not None:
                operands.append(partition_id_tensor())
            return tuple(_bass_exec_p.bind(
                *operands,
                out_avals=tuple(out_avals),
                in_names=tuple(all_names),
                out_names=tuple(out_names),
                lowering_input_output_aliases=(),
                sim_require_finite=False,
                sim_require_nnan=False,
                nc=nc,
            ))

        devices = jax.devices()[:n_cores]
        self.mesh = Mesh(np.asarray(devices), ("core",))
        self.sharding = NamedSharding(self.mesh, PartitionSpec("core"))
        n_io = len(in_names) + len(out_names)
        self.fn = jax.jit(
            shard_map(_body, mesh=self.mesh,
                      in_specs=(PartitionSpec("core"),) * n_io,
                      out_specs=(PartitionSpec("core"),) * len(out_names),
                      check_rep=False),
            keep_unused=True,
        )
        self.resident = {}   # name -> device-resident jax array
        self.zero_outs = None
        self.xkey = None
        self.wkey = None

    def put(self, name, arr):
        self.resident[name] = jax.device_put(arr, self.sharding)

    def ensure_outs(self):
        if self.zero_outs is None:
            self.zero_outs = [
                jax.device_put(
                    np.zeros((self.n_cores * a.shape[0], *a.shape[1:]), a.dtype),
                    self.sharding)
                for a in self.out_avals]

    def dispatch(self):
        """Launch with all-resident args; returns unfetched device outputs."""
        self.ensure_outs()
        args = [self.resident[n] for n in self.in_names] + self.zero_outs
        return self.fn(*args)


def _build_bass(KA, KB, CA, CB, cumA, cumB):
    """One SPMD kernel: x-allgather, L1 gathers+block, h-allgather,
    L2 gathers+block, decoder. KA/KB: per-tile (group-padded) K."""
    nc = bacc.Bacc("TRN2", target_bir_lowering=False, debug=False,
                   num_devices=NCORES, num_swdge_queues=N_SWDGE_Q)
    f32, bf16 = mybir.dt.float32, mybir.dt.bfloat16
    i32 = mybir.dt.int32
    Relu = mybir.ActivationFunctionType.Relu
    Sigmoid = mybir.ActivationFunctionType.Sigmoid
    RG = [list(range(NCORES))]

    x_shard = nc.dram_tensor("x_shard", [NSH, CIN], bf16, kind="ExternalInput")
    Ws1 = nc.dram_tensor("Ws1", [3 * CIN + 1, COUT], f32, kind="ExternalInput")
    Ws2 = nc.dram_tensor("Ws2", [3 * COUT + 1, COUT], f32, kind="ExternalInput")
    Wd1a = nc.dram_tensor("Wd1a", [COUT + 1, COUT], f32, kind="ExternalInput")
    Wd2a = nc.dram_tensor("Wd2a", [COUT + 1, 1], f32, kind="ExternalInput")
    idx_a1 = nc.dram_tensor("idx_a1", [128, CA], i32, kind="ExternalInput")
    idx_b1 = nc.dram_tensor("idx_b1", [128, CB], i32, kind="ExternalInput")
    idx_a2 = nc.dram_tensor("idx_a2", [128, CA], i32, kind="ExternalInput")
    idx_b2 = nc.dram_tensor("idx_b2", [128, CB], i32, kind="ExternalInput")
    own_idx = nc.dram_tensor("own_idx", [128, NT], i32, kind="ExternalInput")
    recip6 = nc.dram_tensor("recip6", [128, NT * CIN], f32, kind="ExternalInput")
    recip32 = nc.dram_tensor("recip32", [128, NT * COUT], f32, kind="ExternalInput")
    dec = nc.dram_tensor("dec", [NPC, 1], bf16, kind="ExternalOutput")

    x_table = nc.dram_tensor("x_table", [N_NODES + 8, CIN], bf16,
                             kind="Internal", addr_space="Shared")
    x_stage = nc.dram_tensor("x_stage", [NSH, CIN], bf16, kind="Internal")
    h_own = nc.dram_tensor("h_own", [NPC, COUT], bf16, kind="Internal")
    h_table = nc.dram_tensor("h_table", [NCORES * NPC + 8, COUT], bf16,
                             kind="Internal", addr_space="Shared")

    with TileContext(nc) as tc:
        with tc.tile_pool(name="const", bufs=1) as cpool, \
             tc.tile_pool(name="sbuf", bufs=2) as pool, \
             tc.tile_pool(name="psum", bufs=1, space="PSUM") as psum:
            ident = cpool.tile([128, 128], f32)
            make_identity(nc, ident[:])
            Ws1_sb = cpool.tile([3 * CIN + 1, COUT], f32)
            nc.sync.dma_start(out=Ws1_sb[:], in_=Ws1[:, :])
            Ws2_sb = cpool.tile([3 * COUT + 1, COUT], f32)
            nc.sync.dma_start(out=Ws2_sb[:], in_=Ws2[:, :])
            Wd1_sb = cpool.tile([COUT + 1, COUT], f32)
            nc.sync.dma_start(out=Wd1_sb[:], in_=Wd1a[:, :])
            Wd2_sb = cpool.tile([COUT + 1, 1], f32)
            nc.sync.dma_start(out=Wd2_sb[:], in_=Wd2a[:, :])
            r6_sb = cpool.tile([128, NT * CIN], f32)
            nc.sync.dma_start(out=r6_sb[:], in_=recip6[:, :])
            r32_sb = cpool.tile([128, NT * COUT], f32)
            nc.sync.dma_start(out=r32_sb[:], in_=recip32[:, :])

            # ---- x all-gather + zero pad row ----
            nc.sync.dma_start(out=x_stage[:, :], in_=x_shard[:, :])
            nc.gpsimd.collective_compute(
                "AllGather", mybir.AluOpType.bypass, RG,
                ins=[x_stage[:, :]], outs=[x_table[0:N_NODES, :]])
            zf = cpool.tile([1, CIN], bf16)
            nc.vector.memset(zf[:], 0.0)
            nc.sync.dma_start(out=x_table[XPAD:XPAD + 1, :], in_=zf[:])
            zb = cpool.tile([1, COUT], bf16)
            nc.vector.memset(zb[:], 0.0)
            nc.sync.dma_start(out=h_table[HPAD:HPAD + 1, :], in_=zb[:])

            # ---- own-x gather (gapped) ----
            oidx = cpool.tile([128, NT], i32)
            nc.sync.dma_start(out=oidx[:], in_=own_idx[:, :])
            xo = cpool.tile([128, NT * WA], bf16)
            _emit_gather(nc, cpool, xo, WA, CIN, oidx, NT, x_table)

            h1own = cpool.tile([128, NT * COUT], f32)

            # ================= layer 1 =================
            for G in range(NG):
                g0 = G * GT
                ca0, cb0 = int(cumA[g0]), int(cumB[g0])
                cols_a = int(cumA[g0 + GT] - ca0)
                cols_b = int(cumB[g0 + GT] - cb0)

                ia = pool.tile([128, cols_a], i32, tag="ia1")
                nc.sync.dma_start(out=ia[:], in_=idx_a1[:, ca0:ca0 + cols_a])
                ga = pool.tile([128, cols_a * WA], bf16, tag="ga1")
                _emit_gather(nc, pool, ga, WA, CIN, ia, cols_a, x_table)

                ib = pool.tile([128, cols_b], i32, tag="ib1")
                nc.sync.dma_start(out=ib[:], in_=idx_b1[:, cb0:cb0 + cols_b])
                gb = pool.tile([128, cols_b * WA], bf16, tag="gb1")
                _emit_gather(nc, pool, gb, WA, CIN, ib, cols_b, x_table)

                U = pool.tile([128, GT * (3 * CIN + 1)], f32, tag="U1")
                Uv = U[:].rearrange("p (t c) -> p t c", c=3 * CIN + 1)
                tmpb = pool.tile([128, GT * CIN], f32, tag="tb1")
                for t in range(GT):
                    j = g0 + t
                    oa, ka = int(cumA[j] - ca0), int(KA[j])
                    nc.vector.tensor_reduce(
                        Uv[:, t, 0:CIN],
                        ga[:, oa * WA:(oa + ka) * WA].rearrange(
                            "p (k w) -> p w k", w=WA),
                        axis=mybir.AxisListType.X, op=mybir.AluOpType.add)
                    ob, kb = int(cumB[j] - cb0), int(KB[j])
                    nc.vector.tensor_reduce(
                        tmpb[:, t * CIN:(t + 1) * CIN],
                        gb[:, ob * WA:(ob + kb) * WA].rearrange(
                            "p (k w) -> p w k", w=WA),
                        axis=mybir.AxisListType.X, op=mybir.AluOpType.add)
                nc.vector.tensor_tensor(
                    out=Uv[:, :, CIN:2 * CIN],
                    in0=tmpb[:].rearrange("p (t c) -> p t c", c=CIN),
                    in1=r6_sb[:, g0 * CIN:(g0 + GT) * CIN].rearrange(
                        "p (t c) -> p t c", c=CIN),
                    op=mybir.AluOpType.mult)
                nc.vector.tensor_copy(
                    Uv[:, :, 2 * CIN:3 * CIN],
                    xo[:].rearrange("p (t w) -> p t w", w=WA)[:, g0:g0 + GT, 0:CIN])
                nc.vector.memset(Uv[:, :, 3 * CIN:3 * CIN + 1], 1.0)

                hbf = pool.tile([128, GT * COUT], bf16, tag="hbf")
                for t in range(GT):
                    j = g0 + t
                    UT_ps = psum.tile([3 * CIN + 1, 128], f32, tag="UT1")
                    nc.tensor.transpose(UT_ps[:], Uv[:, t, :], ident[:])
                    UT = pool.tile([3 * CIN + 1, 128], f32, tag="UT1s")
                    nc.vector.tensor_copy(UT[:], UT_ps[:])
                    h_ps = psum.tile([128, COUT], f32, tag="h1p")
                    nc.tensor.matmul(h_ps[:], lhsT=UT[:], rhs=Ws1_sb[:],
                                     start=True, stop=True)
                    nc.scalar.activation(
                        h1own[:].rearrange("p (t c) -> p t c", c=COUT)[:, j, :],
                        h_ps[:], Relu)
                    nc.vector.tensor_copy(
                        hbf[:].rearrange("p (t c) -> p t c", c=COUT)[:, t, :],
                        h1own[:].rearrange("p (t c) -> p t c", c=COUT)[:, j, :])
                nc.sync.dma_start(
                    out=h_own[g0 * 128:(g0 + GT) * 128, :].rearrange(
                        "(t p) c -> p t c", p=128),
                    in_=hbf[:].rearrange("p (t c) -> p t c", c=COUT))

            # ---- h all-gather ----
            nc.gpsimd.collective_compute(
                "AllGather", mybir.AluOpType.bypass, RG,
                ins=[h_own[:, :]], outs=[h_table[0:NCORES * NPC, :]])

            # ================= layer 2 + decoder =================
            for G in range(NG):
                g0 = G * GT
                ca0, cb0 = int(cumA[g0]), int(cumB[g0])
                cols_a = int(cumA[g0 + GT] - ca0)
                cols_b = int(cumB[g0 + GT] - cb0)

                ia = pool.tile([128, cols_a], i32, tag="ia2")
                nc.sync.dma_start(out=ia[:], in_=idx_a2[:, ca0:ca0 + cols_a])
                ga = pool.tile([128, cols_a * WB], bf16, tag="ga2")
                _emit_gather(nc, pool, ga, WB, COUT, ia, cols_a, h_table)

                ib = pool.tile([128, cols_b], i32, tag="ib2")
                nc.sync.dma_start(out=ib[:], in_=idx_b2[:, cb0:cb0 + cols_b])
                gb = pool.tile([128, cols_b * WB], bf16, tag="gb2")
                _emit_gather(nc, pool, gb, WB, COUT, ib, cols_b, h_table)

                U = pool.tile([128, GT * (3 * COUT + 1)], f32, tag="U2")
                Uv = U[:].rearrange("p (t c) -> p t c", c=3 * COUT + 1)
                tmpb = pool.tile([128, GT * COUT], f32, tag="tb2")
                for t in range(GT):
                    j = g0 + t
                    oa, ka = int(cumA[j] - ca0), int(KA[j])
                    nc.vector.tensor_reduce(
                        Uv[:, t, 0:COUT],
                        ga[:, oa * WB:(oa + ka) * WB].rearrange(
                            "p (k w) -> p w k", w=WB),
                        axis=mybir.AxisListType.X, op=mybir.AluOpType.add)
                    ob, kb = int(cumB[j] - cb0), int(KB[j])
                    nc.vector.tensor_reduce(
                        tmpb[:, t * COUT:(t + 1) * COUT],
                        gb[:, ob * WB:(ob + kb) * WB].rearrange(
                            "p (k w) -> p w k", w=WB),
                        axis=mybir.AxisListType.X, op=mybir.AluOpType.add)
                nc.vector.tensor_tensor(
                    out=Uv[:, :, COUT:2 * COUT],
                    in0=tmpb[:].rearrange("p (t c) -> p t c", c=COUT),
                    in1=r32_sb[:, g0 * COUT:(g0 + GT) * COUT].rearrange(
                        "p (t c) -> p t c", c=COUT),
                    op=mybir.AluOpType.mult)
                nc.vector.tensor_copy(
                    Uv[:, :, 2 * COUT:3 * COUT],
                    h1own[:].rearrange("p (t c) -> p t c", c=COUT)[:, g0:g0 + GT, :])
                nc.vector.memset(Uv[:, :, 3 * COUT:3 * COUT + 1], 1.0)

                h2T = pool.tile([COUT + 1, GT * 128], f32, tag="h2T")
                nc.vector.memset(
                    h2T[:].rearrange("q (t p) -> q t p", p=128)[COUT:COUT + 1, :, :], 1.0)
                zT = pool.tile([COUT + 1, GT * 128], f32, tag="zT")
                nc.vector.memset(
                    zT[:].rearrange("q (t p) -> q t p", p=128)[COUT:COUT + 1, :, :], 1.0)
                og = pool.tile([128, GT], bf16, tag="og")
                for t in range(GT):
                    UT_ps = psum.tile([3 * COUT + 1, 128], f32, tag="UT2")
                    nc.tensor.transpose(UT_ps[:], Uv[:, t, :], ident[:])
                    UT = pool.tile([3 * COUT + 1, 128], f32, tag="UT2s")
                    nc.vector.tensor_copy(UT[:], UT_ps[:])
                    h2T_ps = psum.tile([COUT, 128], f32, tag="h2Tp")
                    nc.tensor.matmul(h2T_ps[:], lhsT=Ws2_sb[:], rhs=UT[:],
                                     start=True, stop=True)
                    nc.scalar.activation(
                        h2T[:].rearrange("q (t p) -> q t p", p=128)[0:COUT, t, :],
                        h2T_ps[:], Relu)
                    zT_ps = psum.tile([COUT, 128], f32, tag="zTp")
                    nc.tensor.matmul(
                        zT_ps[:], lhsT=Wd1_sb[:],
                        rhs=h2T[:].rearrange("q (t p) -> q t p", p=128)[:, t, :],
                        start=True, stop=True)
                    nc.scalar.activation(
                        zT[:].rearrange("q (t p) -> q t p", p=128)[0:COUT, t, :],
                        zT_ps[:], Relu)
                    o_ps = psum.tile([128, 1], f32, tag="op")
                    nc.tensor.matmul(
                        o_ps[:],
                        lhsT=zT[:].rearrange("q (t p) -> q t p", p=128)[:, t, :],
                        rhs=Wd2_sb[:], start=True, stop=True)
                    nc.scalar.activation(og[:, t:t + 1], o_ps[:], Sigmoid)
                nc.sync.dma_start(
                    out=dec[g0 * 128:(g0 + GT) * 128, :].rearrange(
                        "(t p) c -> p t c", p=128),
                    in_=og[:].rearrange("p (t c) -> p t c", c=1))

    nc.compile()
    return nc


def _prep(edge_tp, edge_int):
    deg_a = np.bincount(edge_tp[1], minlength=N_NODES).astype(np.int64)
    deg_b = np.bincount(edge_int[1], minlength=N_NODES).astype(np.int64)
    order = np.lexsort((deg_b, deg_a))
    rank = np.empty(N_NODES, np.int64)
    rank[order] = np.arange(N_NODES)
    gt = rank // 128                       # global tile of node
    p_of = (rank % 128).astype(np.int64)
    core_of = (gt % NCORES).astype(np.int64)
    j_of = (gt // NCORES).astype(np.int64)
    tpos = core_of * NPC + j_of * 128 + p_of

    NRANK = NCORES * NPC
    dega_r = np.zeros(NRANK, np.int64)
    degb_r = np.zeros(NRANK, np.int64)
    dega_r[rank] = deg_a
    degb_r[rank] = deg_b
    # per-tile K (no group padding): each tile row j pads only to its own max
    KA = np.maximum(dega_r.reshape(NT, NCORES * 128).max(1), 1).astype(np.int64)
    KB = np.maximum(degb_r.reshape(NT, NCORES * 128).max(1), 1).astype(np.int64)
    cumA = np.concatenate([[0], np.cumsum(KA)]).astype(np.int64)
    cumB = np.concatenate([[0], np.cumsum(KB)]).astype(np.int64)
    CA, CB = int(cumA[-1]), int(cumB[-1])

    def fill(edges, K, cum, values, pad, width):
        """idx array [NCORES, 128, width]: slot (core,p,col) -> value of src."""
        src = edges[0].astype(np.int64)
        dst = edges[1].astype(np.int64)
        key = tpos[dst]
        o2 = np.argsort(key, kind="stable")
        src_s, key_s = src[o2], key[o2]
        uniq, starts, cnts = np.unique(key_s, return_index=True,
                                       return_counts=True)
        k_idx = np.arange(len(src_s)) - np.repeat(starts, cnts)
        c_s = key_s // NPC
        r_s = key_s % NPC
        j_s = r_s // 128
        p_s = r_s % 128
        col = cum[j_s] + k_idx
        out = np.full((NCORES, 128, width), pad, np.int32)
        out[c_s, p_s, col] = values[src_s]
        return out

    node_id = np.arange(N_NODES)
    ia1 = fill(edge_tp, KA, cumA, node_id, XPAD, CA)
    ib1 = fill(edge_int, KB, cumB, node_id, XPAD, CB)
    ia2 = fill(edge_tp, KA, cumA, tpos, HPAD, CA)
    ib2 = fill(edge_int, KB, cumB, tpos, HPAD, CB)

    own = np.full((NCORES, 128, NT), XPAD, np.int32)
    own[core_of, p_of, j_of] = node_id
    rec = np.ones((NCORES, 128, NT), np.float32)
    rec[core_of, p_of, j_of] = 1.0 / np.maximum(deg_b, 1.0)
    recip6 = np.ascontiguousarray(
        np.broadcast_to(rec[:, :, :, None], (NCORES, 128, NT, CIN))
    ).reshape(NCORES, 128, NT * CIN).astype(np.float32)
    recip32 = np.ascontiguousarray(
        np.broadcast_to(rec[:, :, :, None], (NCORES, 128, NT, COUT))
    ).reshape(NCORES, 128, NT * COUT).astype(np.float32)

    unperm = np.empty(N_NODES, np.int64)   # out[node] = dec_flat[unperm[node]]
    unperm[order] = tpos[order]
    return (KA, KB, CA, CB, cumA, cumB, ia1, ib1, ia2, ib2, own,
            recip6, recip32, unperm)


def _stack_weights(W_self1, b1, W_tp1, W_int1, W_res1,
                   W_self2, b2, W_tp2, W_int2, Wd1, bd1, Wd2, bd2):
    Ws1 = np.zeros((3 * CIN + 1, COUT), np.float32)
    Ws1[0:CIN] = np.asarray(W_tp1)
    Ws1[CIN:2 * CIN] = np.asarray(W_int1)
    Ws1[2 * CIN:3 * CIN] = np.asarray(W_self1) + np.asarray(W_res1)
    Ws1[3 * CIN] = np.asarray(b1)
    Ws2 = np.zeros((3 * COUT + 1, COUT), np.float32)
    Ws2[0:COUT] = np.asarray(W_tp2)
    Ws2[COUT:2 * COUT] = np.asarray(W_int2)
    Ws2[2 * COUT:3 * COUT] = np.asarray(W_self2) + np.eye(COUT, dtype=np.float32)
    Ws2[3 * COUT] = np.asarray(b2)
    Wd1_a = np.zeros((COUT + 1, COUT), np.float32)
    Wd1_a[0:COUT] = np.asarray(Wd1)
    Wd1_a[COUT] = np.asarray(bd1)
    Wd2_a = np.zeros((COUT + 1, 1), np.float32)
    Wd2_a[0:COUT] = np.asarray(Wd2).reshape(COUT, 1)
    Wd2_a[COUT] = np.asarray(bd2).ravel()[0]
    return Ws1, Ws2, Wd1_a, Wd2_a


def kernel(x, edge_tp, edge_int,
           W_self1, b1, W_tp1, W_int1, W_res1,
           W_self2, b2, W_tp2, W_int2,
           Wd1, bd1, Wd2, bd2):
    x = np.ascontiguousarray(np.asarray(x, np.float32))
    edge_tp = np.asarray(edge_tp)
    edge_int = np.asarray(edge_int)
    key = hashlib.sha1(edge_tp[:, ::997].tobytes()
                       + edge_int[:, ::997].tobytes()).hexdigest()
    if key not in _CACHE:
        prep = _prep(edge_tp, edge_int)
        (KA, KB, CA, CB, cumA, cumB, ia1, ib1, ia2, ib2, own,
         recip6, recip32, unperm) = prep
        nc = _build_bass(KA, KB, CA, CB, cumA, cumB)
        ck = _Compiled(nc, NCORES)
        ck.put("idx_a1", ia1.reshape(NCORES * 128, CA))
        ck.put("idx_b1", ib1.reshape(NCORES * 128, CB))
        ck.put("idx_a2", ia2.reshape(NCORES * 128, CA))
        ck.put("idx_b2", ib2.reshape(NCORES * 128, CB))
        ck.put("own_idx", own.reshape(NCORES * 128, NT))
        ck.put("recip6", recip6.reshape(NCORES * 128, NT * CIN))
        ck.put("recip32", recip32.reshape(NCORES * 128, NT * COUT))
        _CACHE[key] = (unperm, ck)
    unperm, ck = _CACHE[key]

    # --- x: content-hashed resident upload (async on miss) ---
    xkey = hashlib.sha1(x).digest()
    if ck.xkey != xkey:
        xb = np.ascontiguousarray(x.astype(BF16)).reshape(NCORES * NSH, CIN)
        ck.put("x_shard", xb)
        ck.xkey = xkey

    # --- weights: content-hashed resident upload (async on miss) ---
    wparts = [np.ascontiguousarray(np.asarray(w, np.float32)) for w in (
        W_self1, b1, W_tp1, W_int1, W_res1, W_self2, b2, W_tp2, W_int2,
        Wd1, bd1, Wd2, bd2)]
    h = hashlib.sha1()
    for w in wparts:
        h.update(w)
    wkey = h.digest()
    if ck.wkey != wkey:
        Ws1, Ws2, Wd1_a, Wd2_a = _stack_weights(*wparts)
        ck.put("Ws1", np.tile(Ws1, (NCORES, 1)))
        ck.put("Ws2", np.tile(Ws2, (NCORES, 1)))
        ck.put("Wd1a", np.tile(Wd1_a, (NCORES, 1)))
        ck.put("Wd2a", np.tile(Wd2_a, (NCORES, 1)))
        ck.wkey = wkey

    # --- single-sync pipeline: (put) -> exec -> fetch ---
    outs = ck.dispatch()
    dec = np.asarray(outs[0]).reshape(NCORES * NPC)
    return dec[unperm].reshape(N_NODES, 1).astype(np.float32)
